# revision 1
# baseline (speedup 1.0000x reference)
"""Trainium2 Bass kernel for nn_BatchPitNorm1d (pairwise Gaussian-CDF KDE + inverse-normal).

Math:  u[b,f] = mean_s Phi((x[b,f] - c[s,f]) / bw[f]),  out = ndtri(u),
       bw = sigmoid(bw_param).

Algorithm: for fixed f, ndtri(u) is a smooth function H_f(x) of x alone, so
instead of B*S*F pairwise Phi evals the kernel:
  1. evaluates the erf-sums g_f(t) at N=24 Chebyshev nodes, sharded
     (4 node-groups) x (2 cdf-sample-halves) over 8 cores — 6 nodes x 1024
     samples per core, one fused ACT erf instruction per node
     (accum_out = free-dim sum, per-partition scale/bias = bandwidth),
  2. AllGathers the raw sums (one 3KB/core collective), adds the halves,
  3. applies ndtri at the nodes (rational(3,3) central branch + deg-10
     log-domain tail polynomial, branchless blend),
  4. fits per-feature even/odd Chebyshev coefficients with one PE matmul,
  5. evaluates at local x via two independent all-STT Clenshaw chains
     (even/odd in w = 2*(x/XDOM)^2 - 1) on DVE.
Truncation error ~1.6e-4; total error vs the f32 reference ~5e-4 max-abs,
below the reference's own f32-vs-f64 noise (~6.8e-4).

Layout: features (F=128) on partitions; x and cdf_data arrive pre-transposed
(feature-major) from the host shard step; output returns feature-major and is
un-transposed on gather.
"""

import math
from contextlib import ExitStack

import numpy as np

import concourse.bass as bass
import concourse.bacc as bacc
import concourse.tile as tile
from concourse import mybir
from concourse import bass_utils

F32 = mybir.dt.float32

N_CORES = 8
B, S, F = 512, 2048, 128
BL = B // N_CORES          # 64 batch rows per core
N_CHEB = 24                # Chebyshev nodes / polynomial order
NGRP = 4                   # node groups (cores 2g, 2g+1 share a node group)
NSPL = 2                   # sample splits (even core: half 0, odd: half 1)
NLOC = N_CHEB // NGRP      # 8 nodes per core
SL = S // NSPL             # 1024 samples per core
XDOM = 4.6                 # Chebyshev domain [-XDOM, XDOM] must cover all x
S_CHUNK = 128              # cdf_data DMA/transpose chunk (partition dim)

# Acklam's ndtri rational approximation (rel err ~1.2e-9 in exact arithmetic).
ACK_A = [-3.969683028665376e+01, 2.209460984245205e+02, -2.759285104469687e+02,
         1.383577518672690e+02, -3.066479806614716e+01, 2.506628277459239e+00]
ACK_B = [-5.447609879822406e+01, 1.615858368580409e+02, -1.556989798598866e+02,
         6.680131188771972e+01, -1.328068155288572e+01]
ACK_C = [-7.784894002430293e-03, -3.223964580411365e-01, -2.400758277161838e+00,
         -2.549732539343734e+00, 4.374664141464968e+00, 2.938163982698783e+00]
ACK_D = [7.784695709041462e-03, 3.224671290700398e-01, 2.445134137142996e+00,
         3.754408661907416e+00]
ACK_PLOW = 0.02425

# Tail branch: ndtri(v) = P((ln v - TAIL_C)/TAIL_H) for v in [1e-10, 0.0245],
# fitted offline (deg 12, max abs err 2.9e-6 in f32 Horner).
TAIL_C = -13.367466545685957
TAIL_H = 9.6583843842545
TAIL_P = [-4.662070379682292, 1.987310755685088, 0.39160273722395583,
          0.14883020862070562, 0.06900459865562841, 0.047597883349676406,
          0.03176661817291077, -0.0067910142231145925, -0.009766979457105394,
          0.019414199950933318, 0.014492288844208983]

# Central branch: ndtri(0.5+q) = q * N(r)/D(r), r = q^2, rational (3,3)
# fitted offline (max rel err 1.0e-5).
CEN_NUM = [-14.41153095969586, 34.82754843726583, -17.684192118918105,
           2.5066372796948575]
CEN_DEN = [-14.220558591278943, 20.063397583232298, -8.101751140071201, 1.0]


def _cheb_nodes():
    th = (np.arange(N_CHEB) + 0.5) * np.pi / N_CHEB
    return (XDOM * np.cos(th)).astype(np.float32), th


def _fit_matrix():
    """Map H-at-nodes -> even/odd coefficients.

    Basis: T_j(w) (j<N/2) and xt*T_j(w) (j<N/2), w = 2*xt^2-1, xt = x/XDOM.
    Returns Cfit[n, k] with columns 0..N/2-1 = beta (even), N/2..N-1 = gamma
    (odd), so alpha = H_nodes^T @ Cfit via the PE matmul.
    """
    _, th = _cheb_nodes()
    xt = np.cos(th)                      # normalized nodes
    w = 2 * xt * xt - 1
    J = N_CHEB // 2
    M = np.zeros((N_CHEB, N_CHEB))
    for j in range(J):
        M[:, j] = np.cos(j * np.arccos(np.clip(w, -1, 1)))
        M[:, J + j] = xt * M[:, j]
    Minv = np.linalg.inv(M)              # coeffs = Minv @ H
    return np.ascontiguousarray(Minv.T).astype(np.float32)


def _tt(nc, pool, in0, in1, op, name, tag=None):
    """Two-tensor op emitted as scalar_tensor_tensor (in0+0) op in1 —
    InstTensorScalarPtr supports the DVE 2x SBUF perf mode, InstTensorTensor
    does not."""
    t = pool.tile([in0.shape[0], in0.shape[1]], F32, name=name, tag=tag or name)
    nc.vector.scalar_tensor_tensor(out=t, in0=in0, scalar=0.0, in1=in1,
                                   op0=mybir.AluOpType.add, op1=op)
    return t


def _horner(nc, pool, r, coeffs, name):
    """Evaluate sum_j coeffs[j] * r^(J-1-j) via STT-fused Horner.

    acc_{j+1} = (acc_j + coeffs[j]) * r   [one scalar_tensor_tensor each],
    then a final tensor_scalar add of coeffs[-1].
    """
    p, w = r.shape[0], r.shape[1]
    acc = pool.tile([p, w], F32, name=f"{name}_h0", tag=f"{name}_h")
    nc.vector.tensor_scalar(out=acc, in0=r, scalar1=float(coeffs[0]), scalar2=None,
                            op0=mybir.AluOpType.mult)
    for j, cj in enumerate(coeffs[1:-1]):
        acc2 = pool.tile([p, w], F32, name=f"{name}_h{j + 1}", tag=f"{name}_h")
        nc.vector.scalar_tensor_tensor(out=acc2, in0=acc, scalar=float(cj),
                                       in1=r, op0=mybir.AluOpType.add,
                                       op1=mybir.AluOpType.mult)
        acc = acc2
    accf = pool.tile([p, w], F32, name=f"{name}_hf", tag=f"{name}_h")
    nc.vector.tensor_scalar(out=accf, in0=acc, scalar1=float(coeffs[-1]),
                            scalar2=None, op0=mybir.AluOpType.add)
    return accf


def _emit_ndtri(nc, pool, g, width, P=128, gscale=1.0):
    """Branchless ndtri(0.5 + g*gscale) on a [P, width] tile of raw erf-sums.

    Central branch: q*N(r)/D(r) rational(3,3); tail: deg-10 polynomial in
    (ln v - C)/H (needs only the Ln table set, no exp/sqrt).  The two DVE
    chains are independent and emitted interleaved; Ln runs on ACT.
    """
    ADD, MUL, SUB = (mybir.AluOpType.add, mybir.AluOpType.mult,
                     mybir.AluOpType.subtract)

    def ts(eng, name, in0, s1, s2=None, op0=MUL, op1=ADD):
        t = pool.tile([P, width], F32, name=name, tag=name)
        if s2 is None:
            eng.tensor_scalar(out=t, in0=in0, scalar1=s1, scalar2=None, op0=op0)
        else:
            eng.tensor_scalar(out=t, in0=in0, scalar1=s1, scalar2=s2,
                              op0=op0, op1=op1)
        return t

    def stt(eng, name, in0, s, in1, op0, op1):
        t = pool.tile([P, width], F32, name=name, tag=name)
        eng.scalar_tensor_tensor(out=t, in0=in0, scalar=s, in1=in1,
                                 op0=op0, op1=op1)
        return t

    def horner(eng, key, xvar, coeffs):
        acc = ts(eng, f"{key}0", xvar, float(coeffs[0]))
        for i, cj in enumerate(coeffs[1:-1]):
            acc = stt(eng, f"{key}{i + 1}", acc, float(cj), xvar, ADD, MUL)
        return ts(eng, f"{key}f", acc, 1.0, float(coeffs[-1]))

    dve, gps = nc.vector, nc.vector

    # prolog (DVE): all four derived ops read g directly (no serial chain)
    u = ts(dve, "ung", g, gscale, 0.5)
    omu = ts(dve, "omu", g, -gscale, 0.5)
    v = stt(dve, "v", u, 1e-10, omu, mybir.AluOpType.max, mybir.AluOpType.min)
    lnv = pool.tile([P, width], F32, name="lnv")
    nc.scalar.activation(out=lnv, in_=v, func=mybir.ActivationFunctionType.Ln)
    mge = ts(dve, "mge", g, 0.0, None, op0=mybir.AluOpType.is_ge)
    nsgn = ts(dve, "nsgn", mge, -2.0, 1.0)

    # two independent DVE chains, emitted interleaved so the per-instruction
    # write-ack pipelines across them
    st = {}
    tai = []
    tai.append(lambda: st.update(m=ts(dve, "mt", lnv, 1.0 / TAIL_H,
                                      -TAIL_C / TAIL_H)))
    tai.append(lambda: st.update(tp=ts(dve, "tp0", st["m"],
                                       float(TAIL_P[::-1][0]))))
    for _i, _cj in enumerate(TAIL_P[::-1][1:-1]):
        tai.append(lambda _cj=_cj, _i=_i: st.update(
            tp=stt(dve, f"tp{_i + 1}", st["tp"], float(_cj), st["m"], ADD, MUL)))
    tai.append(lambda: st.update(tp=ts(dve, "tpf", st["tp"], 1.0,
                                       float(TAIL_P[0]), op0=MUL, op1=ADD)))
    tai.append(lambda: st.update(xt=stt(dve, "xtl", st["tp"], 0.0, nsgn,
                                        ADD, MUL)))

    cen = []
    cen.append(lambda: st.update(q=ts(dve, "qc", g, gscale, None, op0=MUL)))
    cen.append(lambda: st.update(r=stt(dve, "rc", st["q"], 0.0, st["q"],
                                       ADD, MUL)))
    cen.append(lambda: st.update(cn=ts(dve, "cn0", st["r"], float(CEN_NUM[0]))))
    for _i, _cj in enumerate(CEN_NUM[1:-1]):
        cen.append(lambda _cj=_cj, _i=_i: st.update(
            cn=stt(dve, f"cn{_i + 1}", st["cn"], float(_cj), st["r"], ADD, MUL)))
    cen.append(lambda: st.update(cn=ts(dve, "cnf", st["cn"], 1.0,
                                       float(CEN_NUM[-1]), op0=MUL, op1=ADD)))
    cen.append(lambda: st.update(cd=ts(dve, "cd0", st["r"], float(CEN_DEN[0]))))
    for _i, _cj in enumerate(CEN_DEN[1:-1]):
        cen.append(lambda _cj=_cj, _i=_i: st.update(
            cd=stt(dve, f"cd{_i + 1}", st["cd"], float(_cj), st["r"], ADD, MUL)))
    cen.append(lambda: st.update(cd=ts(dve, "cdf_", st["cd"], 1.0,
                                       float(CEN_DEN[-1]), op0=MUL, op1=ADD)))

    def _recip():
        t = pool.tile([P, width], F32, name="cdi")
        dve.reciprocal(out=t, in_=st["cd"])
        st["cdi"] = t
    cen.append(_recip)
    cen.append(lambda: st.update(nq=stt(dve, "nq", st["cn"], 0.0, st["q"],
                                        ADD, MUL)))
    cen.append(lambda: st.update(xc=stt(dve, "xc", st["nq"], 0.0, st["cdi"],
                                        ADD, MUL)))

    while cen or tai:
        if cen:
            cen.pop(0)()
        if tai:
            tai.pop(0)()
    xt, xc = st["xt"], st["xc"]

    # blend: h = xt + [v >= PLOW]*(xc - xt)   (DVE)
    mc = ts(dve, "mcsel", v, float(ACK_PLOW), None, op0=mybir.AluOpType.is_ge)
    d = stt(dve, "dsel", xc, 0.0, xt, ADD, SUB)
    md = stt(dve, "mdsel", mc, 0.0, d, ADD, MUL)
    h = stt(dve, "hout", xt, 0.0, md, ADD, ADD)
    return h


def build(with_collective=True, stages=("load", "grid", "ndtri", "gather", "fit",
                                        "clenshaw", "store"), debug_taps=False,
          repeat=1):
    stages = set(stages)
    nc = bacc.Bacc("TRN2", target_bir_lowering=False, debug=False,
                   enable_asserts=False, num_devices=N_CORES)

    # Inputs arrive pre-transposed (feature-major) from the host shard step.
    x_t = nc.dram_tensor("x_t", [F, BL], F32, kind="ExternalInput")
    cdf_t = nc.dram_tensor("cdf_t", [F, SL], F32, kind="ExternalInput")
    bwp = nc.dram_tensor("bwp", [1, F], F32, kind="ExternalInput")
    tloc = nc.dram_tensor("tloc", [1, NLOC], F32, kind="ExternalInput")
    out = nc.dram_tensor("out", [F, BL], F32, kind="ExternalOutput")
    taps = {}
    if debug_taps:
        for nm, shp in [("d_gacc", [F, NLOC]), ("d_gsum", [N_CHEB, F]),
                        ("d_u", [N_CHEB, F]), ("d_h", [N_CHEB, F]),
                        ("d_alpha", [F, N_CHEB]), ("d_xt1", [F, BL])]:
            taps[nm] = nc.dram_tensor(nm, shp, F32, kind="ExternalOutput")

    cfit_h = nc.inline_tensor(_fit_matrix(), name="cfit")

    with tile.TileContext(nc) as tc, ExitStack() as ctx:
        io = ctx.enter_context(tc.tile_pool(name="io", bufs=2))
        small = ctx.enter_context(tc.tile_pool(name="small", bufs=1))
        nd = ctx.enter_context(tc.tile_pool(name="nd", bufs=3))
        psum = ctx.enter_context(tc.tile_pool(name="psum", bufs=2, space="PSUM"))
        dram = ctx.enter_context(tc.tile_pool(name="dram", bufs=1, space="DRAM"))
        clen = ctx.enter_context(tc.tile_pool(name="clen", bufs=6))

        # --- constants / small inputs
        cfit_sb = small.tile([N_CHEB, N_CHEB], F32)
        nc.scalar.dma_start(out=cfit_sb, in_=cfit_h[:, :])
        bw_col = small.tile([F, 1], F32)
        nc.scalar.dma_start(out=bw_col, in_=bwp.ap().rearrange("o f -> f o"))
        t_bc = small.tile([F, NLOC], F32)
        nc.scalar.dma_start(
            out=t_bc,
            in_=bass.AP(tensor=tloc, offset=0, ap=[[0, F], [1, NLOC]]),
        )

        # --- bandwidth scalars: a = 1/(sigmoid(bwp)*sqrt(2)); neg_a = -a
        bw_sig = small.tile([F, 1], F32)
        nc.scalar.activation(out=bw_sig, in_=bw_col,
                             func=mybir.ActivationFunctionType.Sigmoid)
        inv_bw = small.tile([F, 1], F32)
        nc.vector.reciprocal(out=inv_bw, in_=bw_sig)
        a_col = small.tile([F, 1], F32)
        nc.vector.tensor_scalar(out=a_col, in0=inv_bw, scalar1=1.0 / math.sqrt(2.0),
                                scalar2=None, op0=mybir.AluOpType.mult)
        neg_a = small.tile([F, 1], F32)
        nc.vector.tensor_scalar(out=neg_a, in0=a_col, scalar1=-1.0,
                                scalar2=None, op0=mybir.AluOpType.mult)
        # bias_all[f, j] = a_f * t_j
        bias_all = small.tile([F, NLOC], F32)
        nc.vector.tensor_scalar_mul(out=bias_all, in0=t_bc, scalar1=a_col)

        # --- bulk loads (already feature-major; no transposes needed)
        cT = io.tile([F, SL], F32)
        if "load" in stages:
            nc.sync.dma_start(out=cT, in_=cdf_t[:, :])
        else:
            nc.vector.memset(cT, 0.0)
        x_sb = io.tile([F, BL], F32)
        nc.gpsimd.dma_start(out=x_sb, in_=x_t[:, :])
        xt0 = small.tile([F, BL], F32)
        nc.vector.tensor_scalar(out=xt0, in0=x_sb, scalar1=1.0 / XDOM, scalar2=None,
                                op0=mybir.AluOpType.mult)
        xt1 = small.tile([F, BL], F32)  # clamp to [-1, 1]: off-domain x degrades
        nc.vector.tensor_scalar(out=xt1, in0=xt0, scalar1=1.0, scalar2=-1.0,
                                op0=mybir.AluOpType.min, op1=mybir.AluOpType.max)

        for _rep in range(repeat):
            # --- grid pass: gacc[f, j] = sum_s erf(a_f * (t_j - c_sf)) over the
            # local sample half (ACT, the only O(S) work)
            gacc = nd.tile([F, NLOC], F32, tag="gacc")
            scratch = psum.tile([128, SL], F32, tag="scr", bufs=1)
            if "grid" not in stages:
                nc.vector.memset(gacc, 0.0)
            for j in range(NLOC if "grid" in stages else 0):
                nc.scalar.activation(out=scratch, in_=cT,
                                     func=mybir.ActivationFunctionType.Erf,
                                     bias=bias_all[:, j:j + 1], scale=neg_a,
                                     accum_out=gacc[:, j:j + 1])

            # --- exchange: write gacc^T as [NLOC, F], AllGather (block order =
            # replica rank 2g + h), sum the two sample-halves -> g_sum [N, F]
            cin = dram.tile([NLOC, F], F32, tag=f"cin{_rep}")
            g_sum = nd.tile([N_CHEB, F], F32)
            if "gather" in stages:
                nc.sync.dma_start(out=cin.rearrange("n f -> f n"), in_=gacc)
                cout = dram.tile([N_CORES, NLOC, F], F32,
                                 addr_space="Shared" if with_collective else "Local")
                if with_collective:
                    nc.gpsimd.collective_compute(
                        "AllGather", mybir.AluOpType.bypass,
                        replica_groups=[list(range(N_CORES))],
                        ins=[cin.opt()], outs=[cout.opt()],
                    )
                gh = [nd.tile([N_CHEB, F], F32, name=f"gh{h}", tag=f"gh{h}")
                      for h in range(NSPL)]
                # readback: partition n = g*NLOC + row, skipping over the other half
                for h in range(NSPL):
                    if with_collective:
                        src_ap = bass.AP(
                            tensor=cout.tensor, offset=cout.offset + h * NLOC * F,
                            ap=[[NSPL * NLOC * F, NGRP], [F, NLOC], [1, F]])
                    else:  # stand-in: broadcast-read own block (timing model only)
                        src_ap = bass.AP(tensor=cin.tensor, offset=cin.offset,
                                         ap=[[0, NGRP], [F, NLOC], [1, F]])
                    (nc.scalar if h == 0 else nc.sync).dma_start(
                        out=gh[h][:, :], in_=src_ap)
                nc.vector.scalar_tensor_tensor(
                    out=g_sum, in0=gh[0], scalar=0.0, in1=gh[1],
                    op0=mybir.AluOpType.add, op1=mybir.AluOpType.add)
            else:
                nc.vector.memset(g_sum, 0.0)

            # H = ndtri(0.5 + g/(2S)) on [N, F] (every core, redundantly)
            if "ndtri" in stages:
                h_nodes = _emit_ndtri(nc, nd, g_sum, F, P=N_CHEB,
                                      gscale=1.0 / (2.0 * S))
            else:
                h_nodes = nd.tile([N_CHEB, F], F32, name="u_nodes")
                nc.vector.tensor_scalar(out=h_nodes, in0=g_sum,
                                        scalar1=1.0 / (2.0 * S), scalar2=0.5,
                                        op0=mybir.AluOpType.mult,
                                        op1=mybir.AluOpType.add)

            # --- fit: alpha[f, k] = sum_n H[n, f] * Cfit[n, k]  (one matmul)
            alpha = nd.tile([F, N_CHEB], F32, tag="alpha")
            if "fit" in stages:
                alpha_ps = psum.tile([F, N_CHEB], F32, tag="mm")
                nc.tensor.matmul(out=alpha_ps, lhsT=h_nodes, rhs=cfit_sb,
                                 start=True, stop=True)
                nc.vector.tensor_copy(out=alpha, in_=alpha_ps)
            else:
                nc.vector.memset(alpha, 0.0)

            # --- Clenshaw, even/odd split: y = pe(w) + xt*po(w), w = 2*xt^2-1.
            # Two independent all-STT chains pipeline on DVE without stalling on
            # the per-instruction write-ack.
            xsq = _tt(nc, clen, xt1, xt1, mybir.AluOpType.mult, "xsq")
            wt = clen.tile([F, BL], F32)
            nc.vector.tensor_scalar(out=wt, in0=xsq, scalar1=2.0, scalar2=-1.0,
                                    op0=mybir.AluOpType.mult, op1=mybir.AluOpType.add)
            wt2 = clen.tile([F, BL], F32)
            nc.vector.tensor_scalar(out=wt2, in0=wt, scalar1=2.0, scalar2=None,
                                    op0=mybir.AluOpType.mult)
            J = N_CHEB // 2
            nsteps = J - 1 if "clenshaw" in stages else 0

            def chain(name, col):
                # peeled steps j = J-1 (b=alpha bcast) and J-2 (no subtract)
                b1 = clen.tile([F, BL], F32, name=f"{name}_b0", tag=f"{name}_b")
                nc.vector.tensor_scalar(out=b1, in0=xt1, scalar1=0.0,
                                        scalar2=alpha[:, col + nsteps:col + nsteps + 1],
                                        op0=mybir.AluOpType.mult,
                                        op1=mybir.AluOpType.add)
                b2 = b1
                bn = clen.tile([F, BL], F32, name=f"{name}_c0", tag=f"{name}_c")
                nc.vector.scalar_tensor_tensor(
                    out=bn, in0=wt2, scalar=0.0, in1=b1,
                    op0=mybir.AluOpType.add, op1=mybir.AluOpType.mult)
                bn2 = clen.tile([F, BL], F32, name=f"{name}_b1", tag=f"{name}_b")
                nc.vector.tensor_scalar(out=bn2, in0=bn, scalar1=1.0,
                                        scalar2=alpha[:, col + nsteps - 1:col + nsteps],
                                        op0=mybir.AluOpType.mult,
                                        op1=mybir.AluOpType.add)
                return [bn2, b1]

            ce = chain("ce", 0); co = chain("co", J)
            for j in range(nsteps - 2, 0, -1):
                for name, ch, col in (("ce", ce, j), ("co", co, J + j)):
                    b1, b2 = ch
                    p = clen.tile([F, BL], F32, name=f"{name}_p{j}", tag=f"{name}_p")
                    nc.vector.scalar_tensor_tensor(out=p, in0=b1, scalar=0.0, in1=wt2,
                                                   op0=mybir.AluOpType.add,
                                                   op1=mybir.AluOpType.mult)
                    bn = clen.tile([F, BL], F32, name=f"{name}_b{j}", tag=f"{name}_b")
                    nc.vector.scalar_tensor_tensor(out=bn, in0=p,
                                                   scalar=alpha[:, col:col + 1],
                                                   in1=b2, op0=mybir.AluOpType.add,
                                                   op1=mybir.AluOpType.subtract)
                    ch[1] = b1; ch[0] = bn
            # final step with w (not 2w)
            res = []
            for name, ch, col in (("ce", ce, 0), ("co", co, J)):
                b1, b2 = ch
                p = clen.tile([F, BL], F32, name=f"{name}_pf", tag=f"{name}_p")
                nc.vector.scalar_tensor_tensor(out=p, in0=b1, scalar=0.0, in1=wt,
                                               op0=mybir.AluOpType.add,
                                               op1=mybir.AluOpType.mult)
                r = clen.tile([F, BL], F32, name=f"{name}_r", tag=f"{name}_b")
                nc.vector.scalar_tensor_tensor(out=r, in0=p,
                                               scalar=alpha[:, col:col + 1],
                                               in1=b2, op0=mybir.AluOpType.add,
                                               op1=mybir.AluOpType.subtract)
                res.append(r)
            ye, yo = res
            xyo = _tt(nc, clen, yo, xt1, mybir.AluOpType.mult, "xyo")
            y = _tt(nc, clen, ye, xyo, mybir.AluOpType.add, "yfin")

            # --- store feature-major; the host un-transposes during gather
            nc.sync.dma_start(out=out[:, :], in_=y)

        if debug_taps:
            for h in range(NSPL):
                dt_ = nc.dram_tensor(f"d_gh{h}", [N_CHEB, F], F32,
                                     kind="ExternalOutput")
                nc.sync.dma_start(out=dt_[:, :], in_=gh[h])
            for nm, t in [("d_gacc", gacc), ("d_gsum", g_sum), ("d_u", u_nodes),
                          ("d_h", h_nodes), ("d_alpha", alpha), ("d_xt1", xt1)]:
                nc.sync.dma_start(out=taps[nm][:, :], in_=t)

    nc.compile()
    return nc


_CACHE = {}


def _get_nc():
    if "nc" not in _CACHE:
        _CACHE["nc"] = build(with_collective=True)
    return _CACHE["nc"]


def kernel(x, cdf_data, bw_param):
    x = np.ascontiguousarray(x, dtype=np.float32)
    cdf_data = np.ascontiguousarray(cdf_data, dtype=np.float32)
    bw_param = np.ascontiguousarray(bw_param, dtype=np.float32)
    nc = _get_nc()
    nodes, _ = _cheb_nodes()
    xt = np.ascontiguousarray(x.T)                      # [F, B]
    cdf_halves = [np.ascontiguousarray(cdf_data[h * SL:(h + 1) * SL].T)
                  for h in range(NSPL)]                  # each [F, SL]
    in_maps = []
    for i in range(N_CORES):
        g, h = i // NSPL, i % NSPL
        in_maps.append({
            "x_t": np.ascontiguousarray(xt[:, i * BL:(i + 1) * BL]),
            "cdf_t": cdf_halves[h],
            "bwp": bw_param,
            "tloc": nodes[g * NLOC:(g + 1) * NLOC].reshape(1, NLOC),
        })
    res = bass_utils.run_bass_kernel_spmd(nc, in_maps, core_ids=list(range(N_CORES)))
    return np.concatenate([res.results[i]["out"].T for i in range(N_CORES)], axis=0)



# revision 7
# speedup vs baseline: 1.7408x; 1.7408x over previous
"""Trainium2 Bass kernel for nn_BatchPitNorm1d (pairwise Gaussian-CDF KDE +
inverse-normal transform).

Math:  u[b,f] = mean_s Phi((x[b,f] - c[s,f]) / bw[f]),  out = ndtri(u),
       bw = sigmoid(bw_param).

Algorithm: for fixed f, ndtri(u) is a smooth function H_f(x) of x alone, so
instead of B*S*F pairwise Phi evals the kernel:
  1. evaluates the raw erf-sums g_f(t) at N=12 Chebyshev nodes, sharded
     (4 node-groups) x (2 cdf-sample-halves) over 8 cores -- 3 nodes x 1024
     samples per core, one fused ACT erf instruction per node (fp16 samples,
     accum_out = free-dim sum, per-partition scale/bias from the bandwidth),
  2. AllGathers the raw sums (one small collective), adds the halves,
  3. applies ndtri at the nodes via a per-node degree-4 polynomial in
     ln(min-side mass) -- coefficients fitted offline over each node's
     provable mass window; one ACT Ln + 4 DVE ops total,
  4. converts H-at-nodes to per-feature degree-11 monomial coefficients with
     one fp32r PE matmul that also broadcasts them into scan order,
  5. evaluates the polynomial at local x with tensor_tensor_scan Horner
     segments (state = x*state + coeff, reset by a zero in data0).
Total error vs the f32 reference ~1.2e-3 rel L2 (tolerance 2e-2).

Layout: features (F=128) on partitions; x and cdf_data arrive pre-transposed
(feature-major) from the host shard step; cdf arrives fp16 with an 8-column
header carrying bw_param and the negated local nodes as bitcast f32 pairs.
"""

import math
from contextlib import ExitStack

import numpy as np

import concourse.bass as bass
import concourse.bacc as bacc
import concourse.tile as tile
from concourse import mybir
from concourse import bass_utils

F32 = mybir.dt.float32
F16 = mybir.dt.float16
F32R = mybir.dt.float32r

N_CORES = 8
B, S, F = 512, 2048, 128
BL = B // N_CORES            # 64 batch rows per core
N_CHEB = 12                  # Chebyshev nodes / polynomial order
NGRP = 4                     # node groups (cores 2g, 2g+1 share a group)
NSPL = 2                     # sample splits (even core: half 0, odd: half 1)
NLOC = N_CHEB // NGRP        # 3 nodes per core
SL = S // NSPL               # 1024 samples per core
DEG = 4                      # per-node ndtri poly degree (in ln m)
XDOM = 4.6                   # Chebyshev domain [-XDOM, XDOM] covers all x
HEAD = 8                     # fp16 header columns (p:2, -t:6)
SEG = N_CHEB                 # scan segment length
SCF = SEG * BL               # 768: scan free size
SCH = SCF // 2               # 384: per-matmul / per-scan half

# ---- offline-fitted constants (see gen_consts.py) -------------------------
# fp16-rounded Chebyshev nodes, descending
NODES_T = np.array([4.5625, 4.25, 3.6503906, 2.8007812, 1.7607422, 0.60058594,
                    -0.60058594, -1.7607422, -2.8007812, -3.6503906, -4.25,
                    -4.5625], dtype=np.float32)

# monomial fit matrix: alpha[f, j] = sum_n H[n, f] * CFIT[n, j]
CFIT = np.array([[-0.010885337, -0.010974806, 0.77226454, 0.77861196, -8.349755, -8.418384, 30.470596, 30.721039, -44.12715, -44.48984, 21.870737, 22.050497],
 [0.03445895, 0.037296746, -2.4393632, -2.640252, 26.059607, 28.205692, -92.79864, -100.44088, 129.02852, 139.6544, -60.075264, -65.02264],
 [-0.06395965, -0.08059806, 4.501093, 5.6720033, -46.5262, -58.629486, 155.02762, 195.35637, -195.09756, -245.85008, 82.26224, 103.661865],
 [0.10860824, 0.17837806, -7.5226874, -12.355254, 70.8497, 116.36347, -197.58916, -324.5202, 216.32408, 355.29044, -82.23126, -135.05653],
 [-0.20121995, -0.5256941, 13.106775, 34.241905, -79.401764, -207.43985, 178.21402, 465.5903, -171.8964, -449.08527, 60.211346, 157.30423],
 [0.63299775, 4.848248, -8.418082, -64.47567, 37.368404, 286.2116, -73.32445, -561.60565, 65.76851, 503.73334, -22.037802, -168.79164],
 [0.63299775, -4.848248, -8.418082, 64.47567, 37.368404, -286.2116, -73.32445, 561.60565, 65.76851, -503.73334, -22.037802, 168.79164],
 [-0.20121995, 0.5256941, 13.106775, -34.241905, -79.401764, 207.43985, 178.21402, -465.5903, -171.8964, 449.08527, 60.211346, -157.30423],
 [0.10860824, -0.17837806, -7.5226874, 12.355254, 70.8497, -116.36347, -197.58916, 324.5202, 216.32408, -355.29044, -82.23126, 135.05653],
 [-0.06395965, 0.08059806, 4.501093, -5.6720033, -46.5262, 58.629486, 155.02762, -195.35637, -195.09756, 245.85008, 82.26224, -103.661865],
 [0.03445895, -0.037296746, -2.4393632, 2.640252, 26.059607, -28.205692, -92.79864, 100.44088, 129.02852, -139.6544, -60.075264, 65.02264],
 [-0.010885337, 0.010974806, 0.77226454, -0.77861196, -8.349755, 8.418384, 30.470596, -30.721039, -44.12715, 44.48984, 21.870737, -22.050497]], dtype=np.float32)

# per-node ndtri-in-ln(m) coefficients, highest power first ([c4..c1, c0])
NDTRI_C = np.array([[-3.6171707e-05, -0.0018634178, -0.040726602, -0.6412014, 0.07262636],
 [-5.7895726e-05, -0.0026105207, -0.05006079, -0.6912791, -0.02452162],
 [-0.00013960131, -0.004901463, -0.07361354, -0.79628193, -0.19552192],
 [-0.00057560456, -0.013648459, -0.13895203, -1.0114229, -0.45871168],
 [-0.0044020396, -0.060427357, -0.35491186, -1.4579812, -0.8077913],
 [-0.07646532, -0.5090187, -1.4098656, -2.5751922, -1.260023],
 [0.07646532, 0.5090187, 1.4098656, 2.5751922, 1.260023],
 [0.0044020396, 0.060427357, 0.35491186, 1.4579812, 0.8077913],
 [0.00057560456, 0.013648459, 0.13895203, 1.0114229, 0.45871168],
 [0.00013960131, 0.004901463, 0.07361354, 0.79628193, 0.19552192],
 [5.7895726e-05, 0.0026105207, 0.05006079, 0.6912791, 0.02452162],
 [3.6171707e-05, 0.0018634178, 0.040726602, 0.6412014, -0.07262636]], dtype=np.float32)

# per-node clamp bounds for the raw erf-sum g (fit-window edges in g space)
G_LO = np.array([2002.648, 1983.7097, 1941.2339, 1789.9551, 1244.4845,
                 -302.00018, -1463.7019, -1917.3345, -2037.5852, -2047.6301,
                 -2047.9854, -2047.9982], dtype=np.float32)
G_HI = np.array([2047.9982, 2047.9854, 2047.6301, 2037.5852, 1917.3345,
                 1463.7019, 302.00018, -1244.4845, -1789.9551, -1941.2339,
                 -1983.7097, -2002.648], dtype=np.float32)


def _consts_block():
    """[13, SCF + 8] f32: scan-ordered fit matrix + ndtri chain columns.

    cols 0..SCF-1: CB[n, b*SEG+k] = CFIT[n, SEG-1-k]  (row 12: the folded-in
    ndtri constant terms c0 @ CFIT, paired with an all-ones lhsT row);
    cols SCF..SCF+3: ndtri chain coeffs c4..c1; SCF+4: g_lo; SCF+5: g_hi;
    SCF+6: per-node Ln scale s_j/(2S); SCF+7: pad.
    """
    cb = np.zeros((13, SCF + 8), dtype=np.float32)
    row12 = NDTRI_C[:, DEG] @ CFIT                      # [12]
    for k in range(SEG):
        col = CFIT[:, SEG - 1 - k]
        cb[:12, k::SEG][:, :BL] = col[:, None]
        cb[12, k::SEG][:BL] = row12[SEG - 1 - k]
    cb[:12, SCF:SCF + DEG] = NDTRI_C[:, :DEG]
    cb[:12, SCF + DEG] = G_LO
    cb[:12, SCF + DEG + 1] = G_HI
    sj = np.where(NODES_T <= 0, 1.0, -1.0).astype(np.float32)
    cb[:12, SCF + DEG + 2] = sj / np.float32(2.0 * S)
    cb[:12, SCF + DEG + 3] = 0.5                        # Ln bias column
    return cb


def build(with_collective=True, stages=("load", "grid", "ndtri", "gather",
                                        "fit", "scan", "store")):
    stages = set(stages)
    ADD, MUL, SUB = (mybir.AluOpType.add, mybir.AluOpType.mult,
                     mybir.AluOpType.subtract)
    MIN, MAX = mybir.AluOpType.min, mybir.AluOpType.max
    nc = bacc.Bacc("TRN2", target_bir_lowering=False, debug=False,
                   enable_asserts=False, num_devices=N_CORES)

    cdfh = nc.dram_tensor("cdfh", [F, HEAD + SL], F16, kind="ExternalInput")
    x_t = nc.dram_tensor("x_t", [F, BL], F32, kind="ExternalInput")
    out = nc.dram_tensor("out", [F, BL], F32, kind="ExternalOutput")
    cons_h = nc.inline_tensor(_consts_block(), name="consts")

    with tile.TileContext(nc) as tc, ExitStack() as ctx:
        io = ctx.enter_context(tc.tile_pool(name="io", bufs=2))
        small = ctx.enter_context(tc.tile_pool(name="small", bufs=1))
        nd = ctx.enter_context(tc.tile_pool(name="nd", bufs=2))
        psum = ctx.enter_context(tc.tile_pool(name="psum", bufs=2, space="PSUM"))
        dram = ctx.enter_context(tc.tile_pool(name="dram", bufs=1, space="DRAM"))

        # --- bulk loads.  cdfh first (longest transfer), consts second, x on
        # the Pool SWDGE queue.  ACT SEQ stays clear for table loads.
        cd_sb = io.tile([F, HEAD + SL], F16)
        if "load" in stages:
            nc.sync.dma_start(out=cd_sb, in_=cdfh[:, :])
        else:
            nc.vector.memset(cd_sb, 0.0)
        cons_sb = small.tile([13, SCF + 8], F32)
        nc.sync.dma_start(out=cons_sb, in_=cons_h[:, :])
        x_sb = io.tile([F, BL], F32)
        nc.gpsimd.dma_start(out=x_sb, in_=x_t[:, :])

        # --- bandwidth scalars: neg_a = -1/(sigmoid(p)*sqrt(2)) via the
        # sigmoid table (same ACT table set as erf -> one table load)
        p_col = cd_sb[:, 0:2].bitcast(F32)               # [F, 1]
        negt = cd_sb[:, 2:HEAD].bitcast(F32)             # [F, NLOC] = -t_j
        sig = small.tile([F, 1], F32)
        nc.scalar.activation(out=sig, in_=p_col,
                             func=mybir.ActivationFunctionType.Sigmoid)
        rcp = small.tile([F, 1], F32)
        nc.vector.reciprocal(out=rcp, in_=sig)
        neg_a = small.tile([F, 1], F32)
        nc.vector.tensor_scalar(out=neg_a, in0=rcp,
                                scalar1=-1.0 / math.sqrt(2.0), scalar2=None,
                                op0=MUL)
        bias_all = small.tile([F, NLOC], F32)            # a_f * t_j
        nc.vector.tensor_scalar_mul(out=bias_all, in0=negt, scalar1=neg_a)

        # --- epilogue prep (off critical path): xt = clamp(x/XDOM), and the
        # scan data0 pattern x12[f, b*SEG+k] = 0 if k==0 else xt[f, b]
        xt0 = small.tile([F, BL], F32)
        nc.vector.tensor_scalar(out=xt0, in0=x_sb, scalar1=1.0 / XDOM,
                                scalar2=1.0, op0=MUL, op1=MIN)
        xt1 = small.tile([F, BL], F32)
        nc.vector.tensor_scalar(out=xt1, in0=xt0, scalar1=-1.0, scalar2=None,
                                op0=MAX)
        x12 = small.tile([F, SCF], F32)
        x12_3d = x12.rearrange("f (b k) -> f b k", k=SEG)
        nc.vector.memset(x12_3d[:, :, 0:1], 0.0)
        nc.vector.tensor_copy(out=x12_3d[:, :, 1:SEG],
                              in_=xt1.unsqueeze(2).broadcast_to([F, BL, SEG - 1]))

        # --- grid pass: gacc[f, j] = sum_s erf(neg_a*c + a*t_j) (ACT)
        gacc = small.tile([F, NLOC], F32)
        scratch = psum.tile([128, SL], F32, tag="scr", bufs=1)
        if "grid" in stages:
            for j in range(NLOC):
                nc.scalar.activation(out=scratch, in_=cd_sb[:, HEAD:HEAD + SL],
                                     func=mybir.ActivationFunctionType.Erf,
                                     bias=bias_all[:, j:j + 1], scale=neg_a,
                                     accum_out=gacc[:, j:j + 1])
        else:
            nc.vector.memset(gacc, 0.0)

        # --- exchange: transpose-write local block, AllGather, read all 24
        # (node, half) rows back in one DMA, add halves
        cin = dram.tile([NLOC, F], F32)
        gat = nd.tile([2 * N_CHEB, F], F32)
        if "gather" in stages:
            # split write: nodes 0-1 go as soon as their erf lands
            wa = bass.AP(tensor=cin.tensor, offset=cin.offset,
                         ap=[[1, F], [F, 2]])
            nc.sync.dma_start(out=wa, in_=gacc[:, 0:2])
            wb = bass.AP(tensor=cin.tensor, offset=cin.offset + 2 * F,
                         ap=[[1, F], [F, 1]])
            nc.sync.dma_start(out=wb, in_=gacc[:, 2:3])
            cout = dram.tile([N_CORES, NLOC, F], F32,
                             addr_space="Shared" if with_collective else "Local")
            if with_collective:
                nc.gpsimd.collective_compute(
                    "AllGather", mybir.AluOpType.bypass,
                    replica_groups=[list(range(N_CORES))],
                    ins=[cin.opt()], outs=[cout.opt()],
                )
                # core i holds (g = i % NGRP, h = i // NGRP), so blocks in
                # rank order enumerate h-major: dest partition = 12h + 3g + r
                src_ap = bass.AP(tensor=cout.tensor, offset=cout.offset,
                                 ap=[[NLOC * F, N_CORES], [F, NLOC], [1, F]])
            else:  # stand-in: broadcast-read own block (timing model only)
                src_ap = bass.AP(tensor=cin.tensor, offset=cin.offset,
                                 ap=[[0, N_CORES], [F, NLOC], [1, F]])
            nc.sync.dma_start(out=gat, in_=src_ap)
        else:
            nc.vector.memset(gat, 0.0)

        g = nd.tile([N_CHEB, F], F32)
        nc.vector.scalar_tensor_tensor(out=g, in0=gat[0:N_CHEB, :], scalar=0.0,
                                       in1=gat[N_CHEB:2 * N_CHEB, :],
                                       op0=ADD, op1=ADD)

        # --- ndtri at nodes: clamp g, lnm = Ln(sj/(2S)*g + 0.5), then the
        # per-node chain ((c4*lnm + c3)*lnm + ...)*lnm; c0 is folded into the
        # fit matmul via the all-ones lhsT row 12.
        haug = nd.tile([N_CHEB + 1, F], F32)
        nc.vector.memset(haug[N_CHEB:N_CHEB + 1, :], 1.0)
        ccol = lambda k: cons_sb[0:N_CHEB, SCF + k:SCF + k + 1]  # noqa: E731
        if "ndtri" in stages:
            gcl = nd.tile([N_CHEB, F], F32)
            nc.vector.tensor_scalar(out=gcl, in0=g, scalar1=ccol(DEG + 1),
                                    scalar2=ccol(DEG), op0=MIN, op1=MAX)
            lnm = nd.tile([N_CHEB, F], F32)
            nc.scalar.activation(out=lnm, in_=gcl,
                                 func=mybir.ActivationFunctionType.Ln,
                                 scale=ccol(DEG + 2), bias=ccol(DEG + 3))
            ch = nd.tile([N_CHEB, F], F32, name="ch0", tag="ch")
            nc.vector.tensor_scalar(out=ch, in0=lnm, scalar1=ccol(0),
                                    scalar2=None, op0=MUL)
            for k in range(1, DEG):
                dst = (haug[0:N_CHEB, :] if k == DEG - 1
                       else nd.tile([N_CHEB, F], F32, name=f"ch{k}", tag="ch"))
                nc.vector.scalar_tensor_tensor(out=dst, in0=ch, scalar=ccol(k),
                                               in1=lnm, op0=ADD, op1=MUL)
                ch = dst
        else:
            nc.vector.tensor_copy(out=haug[0:N_CHEB, :], in_=g)

        # --- fit + broadcast: alpha_bcast[f, t] = sum_n Haug[n,f]*CB[n,t],
        # two fp32r matmuls (one PSUM bank each) feeding the two scans
        alpha_ps = [psum.tile([128, SCH], F32, name=f"mm{h}", tag=f"mm{h}")
                    for h in range(2)]
        if "fit" in stages:
            for h in range(2):
                nc.tensor.matmul(out=alpha_ps[h], lhsT=haug.bitcast(F32R),
                                 rhs=cons_sb[:, h * SCH:(h + 1) * SCH].bitcast(F32R),
                                 start=True, stop=True)
        else:
            for h in range(2):
                nc.vector.memset(alpha_ps[h], 0.0)

        # --- Horner scans + strided gather + store (split in two halves so
        # the first store's DGE latency overlaps the second scan)
        for h in range(2):
            bl2 = BL // 2
            scano = small.tile([F, SCH], F32, name=f"scano{h}", tag=f"scano{h}")
            if "scan" in stages:
                nc.vector.tensor_tensor_scan(
                    out=scano, data0=x12[:, h * SCH:(h + 1) * SCH],
                    data1=alpha_ps[h], initial=0.0, op0=MUL, op1=ADD)
            else:
                nc.vector.memset(scano, 0.0)
            y = small.tile([F, bl2], F32, name=f"y{h}", tag=f"y{h}")
            nc.vector.tensor_copy(
                out=y, in_=scano[:, SEG - 1::SEG])
            if "store" in stages:
                nc.sync.dma_start(out=out[:, h * bl2:(h + 1) * bl2], in_=y)

    nc.compile()
    return nc


_CACHE = {}


def _get_nc():
    if "nc" not in _CACHE:
        _CACHE["nc"] = build(with_collective=True)
    return _CACHE["nc"]


def kernel(x, cdf_data, bw_param):
    x = np.ascontiguousarray(x, dtype=np.float32)
    cdf_data = np.ascontiguousarray(cdf_data, dtype=np.float32)
    bw_param = np.ascontiguousarray(bw_param, dtype=np.float32)
    nc = _get_nc()

    xt = np.ascontiguousarray(x.T)                       # [F, B]
    cdf16 = cdf_data.astype(np.float16)
    cdf_halves = [np.ascontiguousarray(cdf16[h * SL:(h + 1) * SL].T)
                  for h in range(NSPL)]                   # each [F, SL] fp16
    p16 = bw_param[0].astype("<f4").view("<f2").reshape(F, 2)  # f32 bit pairs
    in_maps = []
    for i in range(N_CORES):
        g, h = i % NGRP, i // NGRP
        negt = (-NODES_T[g * NLOC:(g + 1) * NLOC]).astype("<f4").view("<f2")
        head = np.concatenate(
            [p16, np.broadcast_to(negt, (F, 2 * NLOC))], axis=1)  # [F, 8]
        cdfh = np.concatenate([head.astype(np.float16), cdf_halves[h]], axis=1)
        in_maps.append({
            "cdfh": np.ascontiguousarray(cdfh),
            "x_t": np.ascontiguousarray(xt[:, i * BL:(i + 1) * BL]),
        })
    res = bass_utils.run_bass_kernel_spmd(nc, in_maps,
                                          core_ids=list(range(N_CORES)))
    return np.concatenate([res.results[i]["out"].T for i in range(N_CORES)],
                          axis=0)


# revision 11
# speedup vs baseline: 1.8323x; 1.0525x over previous
"""Trainium2 Bass kernel for nn_BatchPitNorm1d (pairwise Gaussian-CDF KDE +
inverse-normal transform).

Math:  u[b,f] = mean_s Phi((x[b,f] - c[s,f]) / bw[f]),  out = ndtri(u),
       bw = sigmoid(bw_param).

Algorithm: for fixed f, ndtri(u) is a smooth function H_f(x) of x alone, so
instead of B*S*F pairwise Phi evals the kernel:
  1. evaluates the raw erf-sums g_f(t) at N=12 Chebyshev nodes, sharded
     (4 node-groups) x (2 cdf-sample-halves) over 8 cores -- 3 nodes x 1024
     samples per core, one fused ACT erf instruction per node (fp16 samples,
     accum_out = free-dim sum, per-partition scale/bias from the bandwidth),
  2. AllGathers the raw sums (one small collective), adds the halves,
  3. applies ndtri at the nodes via a per-node degree-4 polynomial in
     ln(min-side mass) -- coefficients fitted offline over each node's
     provable mass window; one ACT Ln + 4 DVE ops total,
  4. converts H-at-nodes to per-feature degree-11 monomial coefficients with
     one fp32r PE matmul that also broadcasts them into scan order,
  5. evaluates the polynomial at local x with tensor_tensor_scan Horner
     segments (state = x*state + coeff, reset by a zero in data0).
Total error vs the f32 reference ~1.2e-3 rel L2 (tolerance 2e-2).

Layout: features (F=128) on partitions; x and cdf_data arrive pre-transposed
(feature-major) from the host shard step; cdf arrives fp16 with an 8-column
header carrying bw_param and the negated local nodes as bitcast f32 pairs.
"""

import math
from contextlib import ExitStack

import numpy as np

import concourse.bass as bass
import concourse.bacc as bacc
import concourse.tile as tile
from concourse import mybir
from concourse import bass_utils

F32 = mybir.dt.float32
F16 = mybir.dt.float16
F32R = mybir.dt.float32r

N_CORES = 8
B, S, F = 512, 2048, 128
BL = B // N_CORES            # 64 batch rows per core
N_CHEB = 12                  # Chebyshev nodes / polynomial order
NGRP = 4                     # node groups (cores 2g, 2g+1 share a group)
NSPL = 2                     # sample splits (even core: half 0, odd: half 1)
NLOC = N_CHEB // NGRP        # 3 nodes per core
SL = S // NSPL               # 1024 samples per core
DEG = 4                      # per-node ndtri poly degree (in ln m)
XDOM = 4.6                   # Chebyshev domain [-XDOM, XDOM] covers all x
HEAD = 8                     # fp16 header columns (p:2, -t:6)
SEG = N_CHEB                 # scan segment length
SCF = SEG * BL               # 768: scan free size
SCH = SCF // 2               # 384: per-matmul / per-scan half

# ---- offline-fitted constants (see gen_consts.py) -------------------------
# fp16-rounded Chebyshev nodes, descending
NODES_T = np.array([4.5625, 4.25, 3.6503906, 2.8007812, 1.7607422, 0.60058594,
                    -0.60058594, -1.7607422, -2.8007812, -3.6503906, -4.25,
                    -4.5625], dtype=np.float32)

# monomial fit matrix: alpha[f, j] = sum_n H[n, f] * CFIT[n, j]
CFIT = np.array([[-0.010885337, -0.010974806, 0.77226454, 0.77861196, -8.349755, -8.418384, 30.470596, 30.721039, -44.12715, -44.48984, 21.870737, 22.050497],
 [0.03445895, 0.037296746, -2.4393632, -2.640252, 26.059607, 28.205692, -92.79864, -100.44088, 129.02852, 139.6544, -60.075264, -65.02264],
 [-0.06395965, -0.08059806, 4.501093, 5.6720033, -46.5262, -58.629486, 155.02762, 195.35637, -195.09756, -245.85008, 82.26224, 103.661865],
 [0.10860824, 0.17837806, -7.5226874, -12.355254, 70.8497, 116.36347, -197.58916, -324.5202, 216.32408, 355.29044, -82.23126, -135.05653],
 [-0.20121995, -0.5256941, 13.106775, 34.241905, -79.401764, -207.43985, 178.21402, 465.5903, -171.8964, -449.08527, 60.211346, 157.30423],
 [0.63299775, 4.848248, -8.418082, -64.47567, 37.368404, 286.2116, -73.32445, -561.60565, 65.76851, 503.73334, -22.037802, -168.79164],
 [0.63299775, -4.848248, -8.418082, 64.47567, 37.368404, -286.2116, -73.32445, 561.60565, 65.76851, -503.73334, -22.037802, 168.79164],
 [-0.20121995, 0.5256941, 13.106775, -34.241905, -79.401764, 207.43985, 178.21402, -465.5903, -171.8964, 449.08527, 60.211346, -157.30423],
 [0.10860824, -0.17837806, -7.5226874, 12.355254, 70.8497, -116.36347, -197.58916, 324.5202, 216.32408, -355.29044, -82.23126, 135.05653],
 [-0.06395965, 0.08059806, 4.501093, -5.6720033, -46.5262, 58.629486, 155.02762, -195.35637, -195.09756, 245.85008, 82.26224, -103.661865],
 [0.03445895, -0.037296746, -2.4393632, 2.640252, 26.059607, -28.205692, -92.79864, 100.44088, 129.02852, -139.6544, -60.075264, 65.02264],
 [-0.010885337, 0.010974806, 0.77226454, -0.77861196, -8.349755, 8.418384, 30.470596, -30.721039, -44.12715, 44.48984, 21.870737, -22.050497]], dtype=np.float32)

# per-node ndtri-in-ln(m) coefficients, highest power first ([c4..c1, c0])
NDTRI_C = np.array([[-3.6171707e-05, -0.0018634178, -0.040726602, -0.6412014, 0.07262636],
 [-5.7895726e-05, -0.0026105207, -0.05006079, -0.6912791, -0.02452162],
 [-0.00013960131, -0.004901463, -0.07361354, -0.79628193, -0.19552192],
 [-0.00057560456, -0.013648459, -0.13895203, -1.0114229, -0.45871168],
 [-0.0044020396, -0.060427357, -0.35491186, -1.4579812, -0.8077913],
 [-0.07646532, -0.5090187, -1.4098656, -2.5751922, -1.260023],
 [0.07646532, 0.5090187, 1.4098656, 2.5751922, 1.260023],
 [0.0044020396, 0.060427357, 0.35491186, 1.4579812, 0.8077913],
 [0.00057560456, 0.013648459, 0.13895203, 1.0114229, 0.45871168],
 [0.00013960131, 0.004901463, 0.07361354, 0.79628193, 0.19552192],
 [5.7895726e-05, 0.0026105207, 0.05006079, 0.6912791, 0.02452162],
 [3.6171707e-05, 0.0018634178, 0.040726602, 0.6412014, -0.07262636]], dtype=np.float32)

# per-node clamp bounds for the raw erf-sum g (fit-window edges in g space)
G_LO = np.array([2002.648, 1983.7097, 1941.2339, 1789.9551, 1244.4845,
                 -302.00018, -1463.7019, -1917.3345, -2037.5852, -2047.6301,
                 -2047.9854, -2047.9982], dtype=np.float32)
G_HI = np.array([2047.9982, 2047.9854, 2047.6301, 2037.5852, 1917.3345,
                 1463.7019, 302.00018, -1244.4845, -1789.9551, -1941.2339,
                 -1983.7097, -2002.648], dtype=np.float32)


def _consts_block():
    """[13, SCF + 8] f32: scan-ordered fit matrix + ndtri chain columns.

    cols 0..SCF-1: CB[n, b*SEG+k] = CFIT[n, SEG-1-k];
    cols SCF..SCF+4: ndtri chain coeffs c4..c1, c0; SCF+5: g_lo; SCF+6: g_hi;
    SCF+7: per-node Ln scale s_j/(2S); SCF+8: Ln bias 0.5; SCF+9..: pad.
    """
    cb = np.zeros((12, SCF + 16), dtype=np.float32)
    for k in range(SEG):
        col = CFIT[:, SEG - 1 - k]
        cb[:12, k:SCF:SEG] = col[:, None]
    cb[:12, SCF:SCF + DEG + 1] = NDTRI_C
    cb[:12, SCF + DEG + 1] = G_LO
    cb[:12, SCF + DEG + 2] = G_HI
    sj = np.where(NODES_T <= 0, 1.0, -1.0).astype(np.float32)
    cb[:12, SCF + DEG + 3] = sj / np.float32(2.0 * S)
    cb[:12, SCF + DEG + 4] = 0.5                        # Ln bias column
    return cb


def build(with_collective=True, stages=("load", "grid", "ndtri", "gather",
                                        "fit", "scan", "store")):
    stages = set(stages)
    ADD, MUL, SUB = (mybir.AluOpType.add, mybir.AluOpType.mult,
                     mybir.AluOpType.subtract)
    MIN, MAX = mybir.AluOpType.min, mybir.AluOpType.max
    nc = bacc.Bacc("TRN2", target_bir_lowering=False, debug=False,
                   enable_asserts=False, num_devices=N_CORES)

    cdfh = nc.dram_tensor("cdfh", [F, HEAD + SL], F16, kind="ExternalInput")
    x_t = nc.dram_tensor("x_t", [F, BL], F32, kind="ExternalInput")
    out = nc.dram_tensor("out", [F, BL], F32, kind="ExternalOutput")
    cons_h = nc.inline_tensor(_consts_block(), name="consts")

    with tile.TileContext(nc) as tc, ExitStack() as ctx:
        io = ctx.enter_context(tc.tile_pool(name="io", bufs=2))
        small = ctx.enter_context(tc.tile_pool(name="small", bufs=1))
        nd = ctx.enter_context(tc.tile_pool(name="nd", bufs=2))
        psum = ctx.enter_context(tc.tile_pool(name="psum", bufs=2, space="PSUM"))
        dram = ctx.enter_context(tc.tile_pool(name="dram", bufs=1, space="DRAM"))

        # --- bulk loads.  cdfh first (longest transfer), consts second, x on
        # the Pool SWDGE queue.  ACT SEQ stays clear for table loads.
        cd_sb = io.tile([F, HEAD + SL], F16)
        if "load" in stages:
            nc.sync.dma_start(out=cd_sb, in_=cdfh[:, :])
        else:
            nc.vector.memset(cd_sb, 0.0)
        cons_sb = small.tile([12, SCF + 16], F32)
        nc.sync.dma_start(out=cons_sb, in_=cons_h[:, :])
        x_sb = io.tile([F, BL], F32)
        nc.gpsimd.dma_start(out=x_sb, in_=x_t[:, :])

        # --- bandwidth scalars: neg_a = -1/(sigmoid(p)*sqrt(2)) via the
        # sigmoid table (same ACT table set as erf -> one table load)
        p_col = cd_sb[:, 0:2].bitcast(F32)               # [F, 1]
        negt = cd_sb[:, 2:HEAD].bitcast(F32)             # [F, NLOC] = -t_j
        sig = small.tile([F, 1], F32)
        nc.scalar.activation(out=sig, in_=p_col,
                             func=mybir.ActivationFunctionType.Sigmoid)
        rcp = small.tile([F, 1], F32)
        nc.vector.reciprocal(out=rcp, in_=sig)
        neg_a = small.tile([F, 1], F32)
        nc.vector.tensor_scalar(out=neg_a, in0=rcp,
                                scalar1=-1.0 / math.sqrt(2.0), scalar2=None,
                                op0=MUL)
        bias_all = small.tile([F, NLOC], F32)            # a_f * t_j
        nc.vector.tensor_scalar_mul(out=bias_all, in0=negt, scalar1=neg_a)

        # --- epilogue prep (off critical path): xt = clamp(x/XDOM), and the
        # scan data0 pattern x12[f, b*SEG+k] = 0 if k==0 else xt[f, b]
        xt0 = small.tile([F, BL], F32)
        nc.vector.tensor_scalar(out=xt0, in0=x_sb, scalar1=1.0 / XDOM,
                                scalar2=1.0, op0=MUL, op1=MIN)
        xt1 = small.tile([F, BL], F32)
        nc.vector.tensor_scalar(out=xt1, in0=xt0, scalar1=-1.0, scalar2=None,
                                op0=MAX)
        x12 = small.tile([F, SCF], F32)
        x12_3d = x12.rearrange("f (b k) -> f b k", k=SEG)
        nc.vector.memset(x12_3d[:, :, 0:1], 0.0)
        nc.vector.tensor_copy(out=x12_3d[:, :, 1:SEG],
                              in_=xt1.unsqueeze(2).broadcast_to([F, BL, SEG - 1]))

        # --- grid pass: gacc[f, j] = sum_s erf(neg_a*c + a*t_j) (ACT)
        gacc = small.tile([F, NLOC], F32)
        scratch = psum.tile([128, SL], F32, tag="scr", bufs=1)
        if "grid" in stages:
            for j in range(NLOC):
                nc.scalar.activation(out=scratch, in_=cd_sb[:, HEAD:HEAD + SL],
                                     func=mybir.ActivationFunctionType.Erf,
                                     bias=bias_all[:, j:j + 1], scale=neg_a,
                                     accum_out=gacc[:, j:j + 1])
        else:
            nc.vector.memset(gacc, 0.0)

        # --- exchange: transpose-write local block, AllGather, read all 24
        # (node, half) rows back in one DMA as [12, 2F], add the halves
        cin = dram.tile([NLOC, F], F32)
        gat = nd.tile([N_CHEB, 2 * F], F32)
        if "gather" in stages:
            # split write: nodes 0-1 go as soon as their erf lands
            wa = bass.AP(tensor=cin.tensor, offset=cin.offset,
                         ap=[[1, F], [F, 2]])
            nc.sync.dma_start(out=wa, in_=gacc[:, 0:2])
            wb = bass.AP(tensor=cin.tensor, offset=cin.offset + 2 * F,
                         ap=[[1, F], [F, 1]])
            nc.sync.dma_start(out=wb, in_=gacc[:, 2:3])
            cout = dram.tile([N_CORES, NLOC, F], F32,
                             addr_space="Shared" if with_collective else "Local")
            if with_collective:
                nc.gpsimd.collective_compute(
                    "AllGather", mybir.AluOpType.bypass,
                    replica_groups=[list(range(N_CORES))],
                    ins=[cin.opt()], outs=[cout.opt()],
                )
                # core i holds (g = i % NGRP, h = i // NGRP): block i=g+4h at
                # offset i*NLOC*F, node n=3g+r row at n*F within the h-major
                # half.  gat[n, hF+f] <- cout[n*F + h*12F + f].
                src_ap = bass.AP(tensor=cout.tensor, offset=cout.offset,
                                 ap=[[F, N_CHEB], [N_CHEB * F, NSPL], [1, F]])
            else:  # stand-in: broadcast-read own block (timing model only)
                src_ap = bass.AP(tensor=cin.tensor, offset=cin.offset,
                                 ap=[[0, N_CHEB], [0, NSPL], [1, F]])
            nc.sync.dma_start(out=gat, in_=src_ap)
        else:
            nc.vector.memset(gat, 0.0)

        g = nd.tile([N_CHEB, F], F32)
        nc.vector.scalar_tensor_tensor(out=g, in0=gat[:, 0:F], scalar=0.0,
                                       in1=gat[:, F:2 * F],
                                       op0=ADD, op1=ADD)

        # --- ndtri at nodes: clamp g, lnm = Ln(sj/(2S)*g + 0.5), then the
        # per-node chain (((c4*lnm + c3)*lnm + c2)*lnm + c1)*lnm + c0
        haug = nd.tile([N_CHEB, F], F32)
        ccol = lambda k: cons_sb[:, SCF + k:SCF + k + 1]  # noqa: E731
        if "ndtri" in stages:
            gcl = nd.tile([N_CHEB, F], F32)
            nc.vector.tensor_scalar(out=gcl, in0=g, scalar1=ccol(DEG + 2),
                                    scalar2=ccol(DEG + 1), op0=MIN, op1=MAX)
            lnm = nd.tile([N_CHEB, F], F32)
            nc.scalar.activation(out=lnm, in_=gcl,
                                 func=mybir.ActivationFunctionType.Ln,
                                 scale=ccol(DEG + 3), bias=ccol(DEG + 4))
            ch = nd.tile([N_CHEB, F], F32, name="ch0", tag="ch")
            nc.vector.tensor_scalar(out=ch, in0=lnm, scalar1=ccol(0),
                                    scalar2=None, op0=MUL)
            for k in range(1, DEG):
                dst = nd.tile([N_CHEB, F], F32, name=f"ch{k}", tag="ch")
                nc.vector.scalar_tensor_tensor(out=dst, in0=ch, scalar=ccol(k),
                                               in1=lnm, op0=ADD, op1=MUL)
                ch = dst
            nc.vector.tensor_scalar(out=haug, in0=ch, scalar1=ccol(DEG),
                                    scalar2=None, op0=ADD)
        else:
            nc.vector.tensor_copy(out=haug, in_=g)

        # --- fit + broadcast: alpha_bcast[f, t] = sum_n Haug[n,f]*CB[n,t],
        # two fp32r matmuls (one PSUM bank each) feeding the two scans
        alpha_ps = [psum.tile([128, SCH], F32, name=f"mm{h}", tag=f"mm{h}")
                    for h in range(2)]
        if "fit" in stages:
            for h in range(2):
                nc.tensor.matmul(out=alpha_ps[h], lhsT=haug.bitcast(F32R),
                                 rhs=cons_sb[:, h * SCH:(h + 1) * SCH].bitcast(F32R),
                                 start=True, stop=True)
        else:
            for h in range(2):
                nc.vector.memset(alpha_ps[h], 0.0)

        # --- Horner scans + strided gather + store (split in two halves so
        # the first store's DGE latency overlaps the second scan)
        for h in range(2):
            bl2 = BL // 2
            scano = small.tile([F, SCH], F32, name=f"scano{h}", tag=f"scano{h}")
            if "scan" in stages:
                nc.vector.tensor_tensor_scan(
                    out=scano, data0=x12[:, h * SCH:(h + 1) * SCH],
                    data1=alpha_ps[h], initial=0.0, op0=MUL, op1=ADD)
            else:
                nc.vector.memset(scano, 0.0)
            y = small.tile([F, bl2], F32, name=f"y{h}", tag=f"y{h}")
            nc.vector.tensor_copy(
                out=y, in_=scano[:, SEG - 1::SEG])
            if "store" in stages:
                nc.sync.dma_start(out=out[:, h * bl2:(h + 1) * bl2], in_=y)

    nc.compile()
    return nc


_CACHE = {}


def _get_nc():
    if "nc" not in _CACHE:
        _CACHE["nc"] = build(with_collective=True)
    return _CACHE["nc"]


def kernel(x, cdf_data, bw_param):
    x = np.ascontiguousarray(x, dtype=np.float32)
    cdf_data = np.ascontiguousarray(cdf_data, dtype=np.float32)
    bw_param = np.ascontiguousarray(bw_param, dtype=np.float32)
    nc = _get_nc()

    xt = np.ascontiguousarray(x.T)                       # [F, B]
    cdf16 = cdf_data.astype(np.float16)
    cdf_halves = [np.ascontiguousarray(cdf16[h * SL:(h + 1) * SL].T)
                  for h in range(NSPL)]                   # each [F, SL] fp16
    p16 = bw_param[0].astype("<f4").view("<f2").reshape(F, 2)  # f32 bit pairs
    in_maps = []
    for i in range(N_CORES):
        g, h = i % NGRP, i // NGRP
        negt = (-NODES_T[g * NLOC:(g + 1) * NLOC]).astype("<f4").view("<f2")
        head = np.concatenate(
            [p16, np.broadcast_to(negt, (F, 2 * NLOC))], axis=1)  # [F, 8]
        cdfh = np.concatenate([head.astype(np.float16), cdf_halves[h]], axis=1)
        in_maps.append({
            "cdfh": np.ascontiguousarray(cdfh),
            "x_t": np.ascontiguousarray(xt[:, i * BL:(i + 1) * BL]),
        })
    res = bass_utils.run_bass_kernel_spmd(nc, in_maps,
                                          core_ids=list(range(N_CORES)))
    return np.concatenate([res.results[i]["out"].T for i in range(N_CORES)],
                          axis=0)


# revision 13
# speedup vs baseline: 2.0839x; 1.1373x over previous
"""Trainium2 Bass kernel for nn_BatchPitNorm1d (pairwise Gaussian-CDF KDE +
inverse-normal transform).

Math:  u[b,f] = mean_s Phi((x[b,f] - c[s,f]) / bw[f]),  out = ndtri(u),
       bw = sigmoid(bw_param).

Algorithm: for fixed f, ndtri(u) is a smooth function H_f(x) of x alone, so
instead of B*S*F pairwise Phi evals the kernel:
  1. evaluates the raw erf-sums g_f(t) at N=8 Chebyshev nodes, sharded
     (4 node-groups) x (2 cdf-sample-halves) over 8 cores -- 2 nodes x 1024
     samples per core, one fused ACT erf instruction per node (fp16 samples,
     accum_out = free-dim sum, per-partition scale/bias from the bandwidth),
  2. AllGathers the raw sums (one small collective), adds the halves,
  3. applies ndtri at the nodes via a per-node degree-4 polynomial in
     ln(min-side mass) -- coefficients fitted offline over each node's
     provable mass window; one ACT Ln + 5 DVE ops total,
  4. converts H-at-nodes to per-feature degree-7 monomial coefficients with
     two fp32r PE matmuls that also broadcast them into scan order,
  5. evaluates the polynomial at local x with tensor_tensor_scan Horner
     segments (state = x*state + coeff, reset by a zero in data0).
Total error vs the f32 reference ~4.5e-3 rel L2 (tolerance 2e-2).

Layout: features (F=128) on partitions; x and cdf_data arrive pre-transposed
(feature-major) from the host shard step; cdf arrives fp16 with an 8-column
header carrying bw_param and the negated local nodes as bitcast f32 pairs.
"""

import math
from contextlib import ExitStack

import numpy as np

import concourse.bass as bass
import concourse.bacc as bacc
import concourse.tile as tile
from concourse import mybir
from concourse import bass_utils

F32 = mybir.dt.float32
F16 = mybir.dt.float16
F32R = mybir.dt.float32r

N_CORES = 8
B, S, F = 512, 2048, 128
BL = B // N_CORES            # 64 batch rows per core
N_CHEB = 8                   # Chebyshev nodes / polynomial order
NGRP = 4                     # node groups (core i: g = i % 4, h = i // 4)
NSPL = 2                     # sample splits
NLOC = N_CHEB // NGRP        # 2 nodes per core
SL = S // NSPL               # 1024 samples per core
DEG = 4                      # per-node ndtri poly degree (in ln m)
XDOM = 4.6                   # Chebyshev domain [-XDOM, XDOM] covers all x
HEAD = 8                     # fp16 header columns (p:2, -t:4, pad:2)
SEG = N_CHEB                 # scan segment length
SCF = SEG * BL               # 512: scan free size
SCH = SCF // 2               # 256: per-matmul / per-scan half

# ---- offline-fitted constants (see gen_consts.py) -------------------------
# fp16-rounded Chebyshev nodes, descending
NODES_T = np.array([4.5117188, 3.8242188, 2.5546875, 0.89746094, -0.89746094,
                    -2.5546875, -3.8242188, -4.5117188], dtype=np.float32)

# monomial fit matrix: alpha[f, j] = sum_n H[n, f] * CFIT[n, j]
CFIT = np.array([[-0.02480779, -0.025293207, 0.76806295, 0.7830917, -3.1724198, -3.234495, 3.0573344, 3.1171575],
 [0.08341817, 0.10034038, -2.548692, -3.0657198, 9.664615, 11.625178, -7.3861294, -8.88448],
 [-0.18711701, -0.33692506, 5.381087, 9.689247, -12.504128, -22.515078, 7.3936634, 13.313116],
 [0.62850666, 3.2214556, -3.600458, -18.454403, 6.0119333, 30.814594, -3.0648682, -15.7092],
 [0.62850666, -3.2214556, -3.600458, 18.454403, 6.0119333, -30.814594, -3.0648682, 15.7092],
 [-0.18711701, 0.33692506, 5.381087, -9.689247, -12.504128, 22.515078, 7.3936634, -13.313116],
 [0.08341817, -0.10034038, -2.548692, 3.0657198, 9.664615, -11.625178, -7.3861294, 8.88448],
 [-0.02480779, 0.025293207, 0.76806295, -0.7830917, -3.1724198, 3.234495, 3.0573344, -3.1171575]], dtype=np.float32)

# per-node ndtri-in-ln(m) coefficients, highest power first ([c4..c1, c0])
NDTRI_C = np.array([[-3.939184e-05, -0.0019820987, -0.04231281, -0.65028685, 0.053851865],
 [-0.0001070687, -0.004050899, -0.06547173, -0.7625096, -0.14434738],
 [-0.0008987558, -0.018857932, -0.1700141, -1.0925689, -0.5370468],
 [-0.033562798, -0.27378634, -0.93934083, -2.1693344, -1.1329428],
 [0.033562798, 0.27378634, 0.93934083, 2.1693344, 1.1329428],
 [0.0008987558, 0.018857932, 0.1700141, 1.0925689, 0.5370468],
 [0.0001070687, 0.004050899, 0.06547173, 0.7625096, 0.14434738],
 [3.939184e-05, 0.0019820987, 0.04231281, 0.65028685, -0.053851865]], dtype=np.float32)

# per-node clamp bounds for the raw erf-sum g (fit-window edges in g space)
G_LO = np.array([1998.2799, 1957.4087, 1719.2457, 175.83878, -1622.4263,
                 -2025.6102, -2047.8419, -2047.9973], dtype=np.float32)
G_HI = np.array([2047.9973, 2047.8419, 2025.6102, 1622.4263, -175.83878,
                 -1719.2457, -1957.4087, -1998.2799], dtype=np.float32)


def _consts_block():
    """[8, SCF + 16] f32: scan-ordered fit matrix + ndtri chain columns.

    cols 0..SCF-1: CB[n, b*SEG+k] = CFIT[n, SEG-1-k];
    cols SCF..SCF+4: ndtri chain coeffs c4..c1, c0; SCF+5: g_lo; SCF+6: g_hi;
    SCF+7: per-node Ln scale s_j/(2S); SCF+8: Ln bias 0.5; rest pad.
    """
    cb = np.zeros((N_CHEB, SCF + 16), dtype=np.float32)
    for k in range(SEG):
        cb[:, k:SCF:SEG] = CFIT[:, SEG - 1 - k][:, None]
    cb[:, SCF:SCF + DEG + 1] = NDTRI_C
    cb[:, SCF + DEG + 1] = G_LO
    cb[:, SCF + DEG + 2] = G_HI
    sj = np.where(NODES_T <= 0, 1.0, -1.0).astype(np.float32)
    cb[:, SCF + DEG + 3] = sj / np.float32(2.0 * S)
    cb[:, SCF + DEG + 4] = 0.5                        # Ln bias column
    return cb


def build(with_collective=True, stages=("load", "grid", "ndtri", "gather",
                                        "fit", "scan", "store")):
    stages = set(stages)
    ADD, MUL = mybir.AluOpType.add, mybir.AluOpType.mult
    MIN, MAX = mybir.AluOpType.min, mybir.AluOpType.max
    nc = bacc.Bacc("TRN2", target_bir_lowering=False, debug=False,
                   enable_asserts=False, num_devices=N_CORES)

    cdfh = nc.dram_tensor("cdfh", [F, HEAD + SL], F16, kind="ExternalInput")
    x_t = nc.dram_tensor("x_t", [F, BL], F32, kind="ExternalInput")
    out = nc.dram_tensor("out", [F, BL], F32, kind="ExternalOutput")
    cons_h = nc.inline_tensor(_consts_block(), name="consts")

    with tile.TileContext(nc) as tc, ExitStack() as ctx:
        io = ctx.enter_context(tc.tile_pool(name="io", bufs=2))
        small = ctx.enter_context(tc.tile_pool(name="small", bufs=1))
        nd = ctx.enter_context(tc.tile_pool(name="nd", bufs=2))
        psum = ctx.enter_context(tc.tile_pool(name="psum", bufs=2, space="PSUM"))
        dram = ctx.enter_context(tc.tile_pool(name="dram", bufs=1, space="DRAM"))

        # --- bulk loads.  cdfh first (longest transfer), consts second, x on
        # the Pool SWDGE queue.  ACT SEQ stays clear for table loads.
        cd_sb = io.tile([F, HEAD + SL], F16)
        if "load" in stages:
            nc.sync.dma_start(out=cd_sb, in_=cdfh[:, :])
        else:
            nc.vector.memset(cd_sb, 0.0)
        cons_sb = small.tile([N_CHEB, SCF + 16], F32)
        nc.sync.dma_start(out=cons_sb, in_=cons_h[:, :])
        x_sb = io.tile([F, BL], F32)
        nc.gpsimd.dma_start(out=x_sb, in_=x_t[:, :])
        # fp32r copy of the scan-ordered fit matrix for the PE matmuls
        cbr = small.tile([N_CHEB, SCF], F32)
        nc.vector.tensor_copy(out=cbr.bitcast(F32R), in_=cons_sb[:, 0:SCF])

        # --- bandwidth scalars: neg_a = -1/(sigmoid(p)*sqrt(2)) via the
        # sigmoid table (same ACT table set as erf -> one table load)
        p_col = cd_sb[:, 0:2].bitcast(F32)               # [F, 1]
        negt = cd_sb[:, 2:2 + 2 * NLOC].bitcast(F32)     # [F, NLOC] = -t_j
        sig = small.tile([F, 1], F32)
        nc.scalar.activation(out=sig, in_=p_col,
                             func=mybir.ActivationFunctionType.Sigmoid)
        rcp = small.tile([F, 1], F32)
        nc.vector.reciprocal(out=rcp, in_=sig)
        neg_a = small.tile([F, 1], F32)
        nc.vector.tensor_scalar(out=neg_a, in0=rcp,
                                scalar1=-1.0 / math.sqrt(2.0), scalar2=None,
                                op0=MUL)
        bias_all = small.tile([F, NLOC], F32)            # a_f * t_j
        nc.vector.tensor_scalar_mul(out=bias_all, in0=negt, scalar1=neg_a)

        # --- epilogue prep on Pool (keeps DVE clear for the bias chain):
        # xt = clamp(x/XDOM); scan data0 x8[f, b*SEG+k] = 0 if k==0 else xt
        xt0 = small.tile([F, BL], F32)
        nc.gpsimd.tensor_scalar(out=xt0, in0=x_sb, scalar1=1.0 / XDOM,
                                scalar2=1.0, op0=MUL, op1=MIN)
        xt1 = small.tile([F, BL], F32)
        nc.gpsimd.tensor_scalar(out=xt1, in0=xt0, scalar1=-1.0, scalar2=None,
                                op0=MAX)
        x8 = small.tile([F, SCF], F32)
        x8_3d = x8.rearrange("f (b k) -> f b k", k=SEG)
        nc.gpsimd.memset(x8_3d[:, :, 0:1], 0.0)
        nc.gpsimd.tensor_copy(out=x8_3d[:, :, 1:SEG],
                              in_=xt1.unsqueeze(2).broadcast_to([F, BL, SEG - 1]))

        # --- grid pass: gacc[f, j] = sum_s erf(neg_a*c + a*t_j) (ACT)
        gacc = small.tile([F, NLOC], F32)
        scratch = psum.tile([128, SL], F32, tag="scr", bufs=1)
        if "grid" in stages:
            for j in range(NLOC):
                nc.scalar.activation(out=scratch, in_=cd_sb[:, HEAD:HEAD + SL],
                                     func=mybir.ActivationFunctionType.Erf,
                                     bias=bias_all[:, j:j + 1], scale=neg_a,
                                     accum_out=gacc[:, j:j + 1])
        else:
            nc.vector.memset(gacc, 0.0)

        # --- exchange: transpose-write local block (one DMA per node, fired
        # as its erf lands), AllGather, read all 16 (node, half) rows back in
        # one DMA as [8, 2F], add the halves
        cin = dram.tile([NLOC, F], F32)
        gat = nd.tile([N_CHEB, 2 * F], F32)
        if "gather" in stages:
            for j in range(NLOC):
                wj = bass.AP(tensor=cin.tensor, offset=cin.offset + j * F,
                             ap=[[1, F], [F, 1]])
                nc.sync.dma_start(out=wj, in_=gacc[:, j:j + 1])
            cout = dram.tile([N_CORES, NLOC, F], F32,
                             addr_space="Shared" if with_collective else "Local")
            if with_collective:
                nc.gpsimd.collective_compute(
                    "AllGather", mybir.AluOpType.bypass,
                    replica_groups=[list(range(N_CORES))],
                    ins=[cin.opt()], outs=[cout.opt()],
                )
                # core i holds (g = i % NGRP, h = i // NGRP): block i=g+4h at
                # offset i*NLOC*F, node n=2g+r row at n*F within the h-major
                # half.  gat[n, hF+f] <- cout[n*F + h*8F + f].
                src_ap = bass.AP(tensor=cout.tensor, offset=cout.offset,
                                 ap=[[F, N_CHEB], [N_CHEB * F, NSPL], [1, F]])
            else:  # stand-in: broadcast-read own block (timing model only)
                src_ap = bass.AP(tensor=cin.tensor, offset=cin.offset,
                                 ap=[[0, N_CHEB], [0, NSPL], [1, F]])
            nc.sync.dma_start(out=gat, in_=src_ap)
        else:
            nc.vector.memset(gat, 0.0)

        g = nd.tile([N_CHEB, F], F32)
        nc.vector.scalar_tensor_tensor(out=g, in0=gat[:, 0:F], scalar=0.0,
                                       in1=gat[:, F:2 * F],
                                       op0=ADD, op1=ADD)

        # warm the PE p-state while the ndtri chain runs (reads g so it fires
        # right after the gather lands, keeping pe_busy_start close)
        warm_ps = psum.tile([N_CHEB, N_CHEB], F32, tag="warm")
        nc.tensor.matmul(out=warm_ps, lhsT=g[:, 0:N_CHEB], rhs=g[:, 0:N_CHEB],
                         start=True, stop=True)

        # --- ndtri at nodes: clamp g, lnm = Ln(sj/(2S)*g + 0.5), then the
        # per-node chain (((c4*lnm + c3)*lnm + c2)*lnm + c1)*lnm + c0
        haug = nd.tile([N_CHEB, F], F32)
        ccol = lambda k: cons_sb[:, SCF + k:SCF + k + 1]  # noqa: E731
        if "ndtri" in stages:
            gcl = nd.tile([N_CHEB, F], F32)
            nc.vector.tensor_scalar(out=gcl, in0=g, scalar1=ccol(DEG + 2),
                                    scalar2=ccol(DEG + 1), op0=MIN, op1=MAX)
            lnm = nd.tile([N_CHEB, F], F32)
            nc.scalar.activation(out=lnm, in_=gcl,
                                 func=mybir.ActivationFunctionType.Ln,
                                 scale=ccol(DEG + 3), bias=ccol(DEG + 4))
            ch = nd.tile([N_CHEB, F], F32, name="ch0", tag="ch")
            nc.vector.tensor_scalar(out=ch, in0=lnm, scalar1=ccol(0),
                                    scalar2=None, op0=MUL)
            for k in range(1, DEG):
                dst = nd.tile([N_CHEB, F], F32, name=f"ch{k}", tag="ch")
                nc.vector.scalar_tensor_tensor(out=dst, in0=ch, scalar=ccol(k),
                                               in1=lnm, op0=ADD, op1=MUL)
                ch = dst
            nc.vector.tensor_scalar(out=haug.bitcast(F32R), in0=ch,
                                    scalar1=ccol(DEG), scalar2=None, op0=ADD)
        else:
            nc.vector.tensor_copy(out=haug.bitcast(F32R), in_=g)

        # --- fit + broadcast: alpha_bcast[f, t] = sum_n H[n,f]*CB[n,t],
        # two fp32r matmuls (one PSUM bank each) feeding the two scans
        alpha_ps = [psum.tile([128, SCH], F32, name=f"mm{h}", tag=f"mm{h}")
                    for h in range(2)]
        if "fit" in stages:
            for h in range(2):
                nc.tensor.matmul(out=alpha_ps[h], lhsT=haug.bitcast(F32R),
                                 rhs=cbr.bitcast(F32R)[:, h * SCH:(h + 1) * SCH],
                                 start=True, stop=True)
        else:
            for h in range(2):
                nc.vector.memset(alpha_ps[h], 0.0)

        # --- Horner scans + strided gather + store (split in two halves so
        # the first store's DGE latency overlaps the second scan)
        for h in range(2):
            bl2 = BL // 2
            scano = small.tile([F, SCH], F32, name=f"scano{h}", tag=f"scano{h}")
            if "scan" in stages:
                nc.vector.tensor_tensor_scan(
                    out=scano, data0=x8[:, h * SCH:(h + 1) * SCH],
                    data1=alpha_ps[h], initial=0.0, op0=MUL, op1=ADD)
            else:
                nc.vector.memset(scano, 0.0)
            y = small.tile([F, bl2], F32, name=f"y{h}", tag=f"y{h}")
            nc.vector.tensor_copy(
                out=y, in_=scano[:, SEG - 1::SEG])
            if "store" in stages:
                nc.sync.dma_start(out=out[:, h * bl2:(h + 1) * bl2], in_=y)

    nc.compile()
    return nc


_CACHE = {}


def _get_nc():
    if "nc" not in _CACHE:
        _CACHE["nc"] = build(with_collective=True)
    return _CACHE["nc"]


def kernel(x, cdf_data, bw_param):
    x = np.ascontiguousarray(x, dtype=np.float32)
    cdf_data = np.ascontiguousarray(cdf_data, dtype=np.float32)
    bw_param = np.ascontiguousarray(bw_param, dtype=np.float32)
    nc = _get_nc()

    xt = np.ascontiguousarray(x.T)                       # [F, B]
    cdf16 = cdf_data.astype(np.float16)
    cdf_halves = [np.ascontiguousarray(cdf16[h * SL:(h + 1) * SL].T)
                  for h in range(NSPL)]                   # each [F, SL] fp16
    p16 = bw_param[0].astype("<f4").view("<f2").reshape(F, 2)  # f32 bit pairs
    in_maps = []
    for i in range(N_CORES):
        g, h = i % NGRP, i // NGRP
        negt = (-NODES_T[g * NLOC:(g + 1) * NLOC]).astype("<f4").view("<f2")
        head = np.zeros((F, HEAD), dtype=np.float16)
        head[:, 0:2] = p16
        head[:, 2:2 + 2 * NLOC] = negt[None, :]
        cdfh = np.concatenate([head, cdf_halves[h]], axis=1)
        in_maps.append({
            "cdfh": np.ascontiguousarray(cdfh),
            "x_t": np.ascontiguousarray(xt[:, i * BL:(i + 1) * BL]),
        })
    res = bass_utils.run_bass_kernel_spmd(nc, in_maps,
                                          core_ids=list(range(N_CORES)))
    return np.concatenate([res.results[i]["out"].T for i in range(N_CORES)],
                          axis=0)
